# revision 48
# baseline (speedup 1.0000x reference)
"""Trainium2 Bass kernel for the AudNet 4-layer LIF spiking network.

Reference computation (per time step t of 81, batch 4096):
    s1, m1 = lif(x_t @ w1.T + b1, m1)     # 129 -> 1000
    s2, m2 = lif(s1 @ w2.T + b2, m2)      # 1000 -> 1000
    s3, m3 = lif(s2 @ w3.T + b3, m3)      # 1000 -> 20
    s4, m4 = lif(s3 @ w4.T + b4, m4)      # 20 -> 10
with lif: reset = (m > 1); m' = 0.95*m + cur - reset; spk = (m' > 1)
Outputs: (s4, m4) per step -> each [81, 4096, 10].

Strategy (v2 — mixed-precision, PE-minimal):
- Data parallel over 8 NeuronCores: 512 batch rows per core; weights
  replicated; no cross-device traffic.
- Hidden-on-partition, batch-on-free layout: weights are the stationary
  lhsT, spikes/x the moving rhs; the 81-step scan needs no transposes.
- fp32 matmuls cost 4 PE cycles/column on TRN2; fp32r (fp32 rounded to
  12 significand bits) costs 1; fp8 in DoubleRow perf mode costs 0.5
  while contracting TWO 128-row k-tiles per instruction.
- Precision budget (numpy-simulated against the fp32 oracle; the spike
  avalanche tolerates ~tens of layer-4 flips in 3.3M):
  * L1 (x/w1) is flip-critical: full hi/lo fp32r scheme, 4 matmuls per
    hidden tile (wh@xh + wh@xl + wl@xh + K=5 combo with f129/bias rows).
  * L2 w2: fp32r hi term (12 bits, 64 matmuls) + the quantization
    residual scaled by 2^20 in fp8e4m3 DoubleRow (4 more bits, 32
    half-cost matmuls) -> ~16.3-bit weights, 12 flips in sim.
  * L3/L4 w3/w4: single fp32r term (8+1 matmuls), 0 extra flips; biases
    get hi+lo rows in spare k-rows (K=127 / K=22).
- Biases ride the contraction dim via ones-rows in the spike tiles.
- LIF per layer per step is 3 elementwise passes, spread over engines:
    opA: m += psum (DVE tt / stt for the scaled fp8 psum)
    opB: spk = m > 1 (GPSIMD is_gt, SBUF-only)
    opC: m = beta*m - spk (DVE scalar_tensor_tensor)
  plus one ACT copy of s1 -> fp8 rhs tile per step, and ACT psum->SBUF
  staging for the small L3/L4 tiles (GPSIMD does their adds).
- Software pipelining as in v1: layer-1 psums for step t+1 run between
  layers 2 and 3 of step t; layer 4 of step t runs inside iteration t+1.
"""

import os
import sys

import numpy as np

for _p in ("/opt/trn_rl_repo", "/root/.axon_site/_ro/trn_rl_repo"):
    if os.path.isdir(_p) and _p not in sys.path:
        sys.path.insert(0, _p)

import ml_dtypes

import concourse.bacc as bacc
import concourse.bass as bass
import concourse.mybir as mybir
import concourse.tile as tile
from concourse.bass_utils import run_bass_kernel_spmd
from concourse.tile_rust import add_dep_helper

# Problem constants (hardcoded; kernel.py must be self-contained).
T = 81          # time steps
F = 129         # input features per step
H = 1000        # hidden units (layers 1, 2)
HT = 125        # hidden tile rows  (H = 8 * 125)
NH = 8          # number of hidden tiles
H3 = 20         # layer-3 units
H4 = 10         # output units
BATCH = 4096
NCORES = 8
B = BATCH // NCORES   # 512 batch rows per core
BETA = 0.95
THRESH = 1.0
XR = 2 * 128 + 5      # x_aug rows: xh[0:128], xl[0:128], 5 combo rows
LO_SCALE = float(2.0 ** 20)    # fp8 residual pre-scale
LO_INV = float(2.0 ** -20)

F32 = mybir.dt.float32
F32R = mybir.dt.float32r
FP8 = mybir.dt.float8e4
AOP = mybir.AluOpType
AFT = mybir.ActivationFunctionType
DR = mybir.MatmulPerfMode.DoubleRow


def build_bass():
    # Bacc (not raw Bass): its compile() runs generate_event_semaphores /
    # move_matmul_waits_to_ldweights, required because TRN2 Matmult
    # instructions can carry at most one sync wait.
    nc = bacc.Bacc(trn_type="TRN2", target_bir_lowering=False)

    x_d = nc.dram_tensor("x_aug", [T, XR, B], F32R, kind="ExternalInput")
    w1h_d = nc.dram_tensor("w1h", [128, H], F32R, kind="ExternalInput")
    w1l_d = nc.dram_tensor("w1l", [128, H], F32R, kind="ExternalInput")
    w1c_d = nc.dram_tensor("w1c", [5, H], F32R, kind="ExternalInput")
    # w2 hi term: chunk k rows 0:125 = rne12(w2T[125k:125k+125]);
    # chunk 7 row 125 = rne12(b2)
    w2h_d = nc.dram_tensor("w2h", [NH, HT + 1, H], F32R, kind="ExternalInput")
    # w2 residual * 2^20 in e4m3, DoubleRow layout [k-row, chunk, out]:
    # rows 125..127 zero except [125, 7, :] = e4m3((b2 - b2h) * 2^20).
    # Out tile h sits at cols h*128 (stride padded so the chunk-pair step
    # is 16B-aligned and tile offsets are 8B-aligned, per the walrus
    # dual-fp8 Ldweights rules). NOTE: a pure multi-term fp8 scheme (no
    # fp32r hi) fails on real HW — the fp8 psum accumulation has limited
    # mantissa, so mixed-magnitude terms lose their small tails.
    HP = 1024
    w2lo_d = nc.dram_tensor("w2lo8", [128, NH, HP], FP8, kind="ExternalInput")
    # w3 single fp32r term, K=127 per chunk: rows 0:125 weights, chunk 7
    # rows 125/126 = b3 hi/lo, zero elsewhere
    w3c_d = nc.dram_tensor("w3c", [NH, HT + 2, H3], F32R, kind="ExternalInput")
    # w4 single fp32r term: rows 0:20 = rne12(w4T), rows 20/21 = b4 hi/lo
    w4c_d = nc.dram_tensor("w4c", [H3 + 2, H4], F32R, kind="ExternalInput")
    # init image for the fp8 spike tile rows 125..127 (ones row + zeros)
    s8i_d = nc.dram_tensor("s18init", [3, NH, B], FP8, kind="ExternalInput")
    outs_d = nc.dram_tensor("out_s", [T, H4, B], F32, kind="ExternalOutput")
    outm_d = nc.dram_tensor("out_m", [T, H4, B], F32, kind="ExternalOutput")

    with tile.TileContext(nc) as tc:
        with (
            tc.tile_pool(name="pers", bufs=1) as pers,
            tc.tile_pool(name="xpool", bufs=3) as xpool,
            tc.tile_pool(name="ps1", bufs=2, space="PSUM") as ps1,
            tc.tile_pool(name="ps2", bufs=3, space="PSUM") as ps2,
            tc.tile_pool(name="pslo", bufs=2, space="PSUM") as pslo,
            tc.tile_pool(name="ps34", bufs=1, space="PSUM") as ps34,
        ):
            # ---- persistent SBUF tensors ----
            w1h = pers.tile([128, H], F32R, tag="w1h")
            w1l = pers.tile([128, H], F32R, tag="w1l")
            w1c = pers.tile([5, H], F32R, tag="w1c")
            w2h = pers.tile([HT + 1, NH * H], F32R, tag="w2h")   # [126, 8000]
            w2lo = pers.tile([128, NH, HP], FP8, tag="w2lo")     # DoubleRow
            w3c = pers.tile([HT + 2, NH * H3], F32R, tag="w3c")  # [127, 160]
            w4c = pers.tile([H3 + 2, H4], F32R, tag="w4c")       # [22, 10]
            m1 = pers.tile([HT, NH * B], F32, tag="m1")          # [125, 4096]
            m2 = pers.tile([HT, NH * B], F32, tag="m2")
            m3 = pers.tile([H3, B], F32, tag="m3")               # [20, 512]
            m4 = pers.tile([H4, B], F32, tag="m4")               # [10, 512]
            s1 = pers.tile([HT + 1, NH * B], F32R, tag="s1")     # [126, 4096]
            s18 = pers.tile([128, NH, B], FP8, tag="s18")        # fp8 rhs
            s2 = pers.tile([HT + 2, NH * B], F32R, tag="s2")     # [127, 4096]
            s3 = pers.tile([H3 + 2, B], F32R, tag="s3")          # [22, 512]
            s4 = pers.tile([H4, B], F32, tag="s4")
            cur3 = pers.tile([H3, B], F32, tag="cur3")           # psum staging
            cur4 = pers.tile([H4, B], F32, tag="cur4")

            # fp32 views of the fp32r spike tiles for elementwise consumers
            s1f = s1[:].bitcast(F32)
            s2f = s2[:].bitcast(F32)
            s3f = s3[:].bitcast(F32)

            # ---- weight loads (layer-1 weights + x(0) first: they gate
            # step 0; the bulk w2/w3/w4 transfers follow) ----
            def load_x(t):
                xh = xpool.tile([128, B], F32R, tag="xh", name="xh")
                xl = xpool.tile([128, B], F32R, tag="xl", name="xl")
                xc = xpool.tile([5, B], F32R, tag="xc", name="xc")
                nc.sync.dma_start(xh[:], x_d[t, 0:128, :])
                nc.sync.dma_start(xl[:], x_d[t, 128:256, :])
                nc.sync.dma_start(xc[:], x_d[t, 256:261, :])
                return xh, xl, xc

            # w1 loads split per hidden tile so the first L1 matmul only
            # waits for its own 64KB slice, not the whole 500KB tensor
            w1_tile_dmas = {h: [] for h in range(NH)}
            for sb, dr_ in [(w1h, w1h_d), (w1l, w1l_d)]:
                for h in range(NH):
                    cs = slice(h * HT, (h + 1) * HT)
                    w1_tile_dmas[h].append(
                        nc.sync.dma_start(sb[:, cs], dr_[:, cs]))
            w1_tile_dmas[0].append(nc.sync.dma_start(w1c[:], w1c_d[:]))
            x0 = load_x(0)
            wdmas = [nc.sync.dma_start(w4c[:], w4c_d[:]),
                     nc.sync.dma_start(w2lo[:], w2lo_d[:]),
                     nc.sync.dma_start(s18[125:128, :, :], s8i_d[:])]
            for k in range(NH):
                wdmas.append(nc.sync.dma_start(
                    w2h[:, k * H:(k + 1) * H], w2h_d[k]))
                wdmas.append(nc.sync.dma_start(
                    w3c[:, k * H3:(k + 1) * H3], w3c_d[k]))

            # Matmult instructions can carry at most ONE sync wait in the
            # TRN2 ISA (fp32/fp32r fuse the weight load into the matmul), so
            # have PE nops absorb the weight-DMA waits before any matmul.
            def absorb(dmas):
                nops = []
                for d in dmas:
                    nop = nc.tensor.nop(nofuse=True)
                    add_dep_helper(nop.ins, d.ins, sync=True,
                                   reason="absorb weight-DMA wait on PE")
                    nops.append(nop)
                return nops

            # ---- state init ----
            nc.vector.memset(m1[:], 0.0)
            nc.vector.memset(m2[:], 0.0)
            nc.gpsimd.memset(m3[:], 0.0)
            nc.gpsimd.memset(m4[:], 0.0)
            # ones rows feeding the bias fold. Engine ops need partition
            # bases in {0,32,64,96}, so memset a wider aligned region; the
            # spike rows below 125 are overwritten by the per-step spike
            # writes before any matmul reads them.
            nc.vector.memset(s1f[96:HT + 1, (NH - 1) * B:], 1.0)
            nc.vector.memset(s2f[96:HT + 2, :], 1.0)   # rows 125/126 ones
            nc.gpsimd.memset(s3f[:, :], 1.0)           # rows 20/21 ones

            def l1_block(xh, xl, xc, tile_dmas=None):
                """Layer-1 psums + LIF opA/opB for one step, per hidden
                tile. opA on DVE, opB (threshold) on GPSIMD. tile_dmas
                (prologue only) maps tile h -> weight DMAs whose waits get
                absorbed by PE nops emitted right before tile h's first
                matmul, so PE starts once tile 0's 64KB weight slice lands
                instead of the whole w1 tensor."""
                first_mm = None
                prev_mm = None
                for h in range(NH):
                    p1 = ps1.tile([HT, B], F32, tag="p1")
                    c0 = h * HT
                    nops = []
                    if tile_dmas:
                        for d in tile_dmas.get(h, []):
                            nop = nc.tensor.nop(nofuse=True)
                            add_dep_helper(nop.ins, d.ins, sync=True,
                                           reason="absorb w1 slice dma")
                            if prev_mm is not None:
                                add_dep_helper(nop.ins, prev_mm.ins,
                                               sync=False,
                                               reason="keep nop after prior tile")
                            nops.append(nop)
                    mm = nc.tensor.matmul(p1[:], w1h[:, c0:c0 + HT], xh[:],
                                          start=True, stop=False)
                    for nop in nops:
                        add_dep_helper(mm.ins, nop.ins, sync=False,
                                       reason="w1 tile absorber order")
                    if first_mm is None:
                        first_mm = mm
                    nc.tensor.matmul(p1[:], w1h[:, c0:c0 + HT], xl[:],
                                     start=False, stop=False)
                    nc.tensor.matmul(p1[:], w1l[:, c0:c0 + HT], xh[:],
                                     start=False, stop=False)
                    prev_mm = nc.tensor.matmul(p1[:], w1c[:, c0:c0 + HT],
                                               xc[:], start=False, stop=True)
                    cols = slice(h * B, (h + 1) * B)
                    nc.vector.tensor_tensor(m1[:, cols], p1[:], m1[:, cols],
                                            AOP.add)
                    nc.gpsimd.tensor_scalar(s1[0:HT, cols], m1[:, cols],
                                            THRESH, None, AOP.is_gt)
                return first_mm

            def l4_block(t):
                """Layer 4 for step t + LIF + output DMAs. Shares the ps34
                bank with layer 3 at partition 0 (fp32r matmuls require
                dst start_partition 0); the bufs=1 rotation serializes."""
                p4 = ps34.tile([H4, B], F32, tag="p34", name="p4")
                nc.tensor.matmul(p4[:], w4c[:], s3[:], start=True, stop=True)
                nc.scalar.copy(cur4[:], p4[:])
                nc.gpsimd.tensor_tensor(m4[:], cur4[:], m4[:], AOP.add)
                nc.sync.dma_start(outm_d[t], m4[:])
                nc.gpsimd.tensor_scalar(s4[:], m4[:], THRESH, None, AOP.is_gt)
                nc.sync.dma_start(outs_d[t], s4[:])
                nc.vector.scalar_tensor_tensor(m4[:], m4[:], BETA, s4[:],
                                               AOP.mult, AOP.subtract)

            # ---- prologue: step 0 layer-1 + fp8 spike image ----
            l1_block(*x0, tile_dmas=w1_tile_dmas)
            # s18 rows 0:125 <- s1 spikes (fp8 cast; 0/1 exact)
            nc.scalar.copy(s18[0:HT, :, :], s1f[0:HT, :])
            nc.vector.scalar_tensor_tensor(m1[:], m1[:], BETA, s1f[0:HT, :],
                                           AOP.mult, AOP.subtract)

            # absorb the remaining weight DMAs before the main loop's
            # layer-2/3/4 matmuls
            late_absorbers = absorb(wdmas)

            # ---- main loop over steps ----
            for i in range(T):
                if i < T - 1:
                    xh, xl, xc = load_x(i + 1)

                # layer-2 hi term of step i: 8 fp32r k-chunks per out tile
                for h in range(NH):
                    p2 = ps2.tile([HT, B], F32, tag="p2")
                    c0 = h * HT
                    for k in range(NH):
                        kk = HT + 1 if k == NH - 1 else HT
                        mm2 = nc.tensor.matmul(
                            p2[:],
                            w2h[0:kk, k * H + c0:k * H + c0 + HT],
                            s1[0:kk, k * B:(k + 1) * B],
                            start=(k == 0), stop=(k == NH - 1))
                        if i == 0 and h == 0 and k == 0:
                            for nop in late_absorbers:
                                add_dep_helper(
                                    mm2.ins, nop.ins, sync=False,
                                    reason="absorbers before first L2 mm")
                    cols = slice(h * B, (h + 1) * B)
                    nc.vector.tensor_tensor(m2[:, cols], p2[:], m2[:, cols],
                                            AOP.add)

                # layer-2 fp8 residual term: 4 DoubleRow chunk-pairs per
                # out tile, each contracting 2x128 k-rows at half cost
                for h in range(NH):
                    plo = pslo.tile([HT, B], F32, tag="plo")
                    c0 = h * 128
                    for p in range(4):
                        nc.tensor.matmul(
                            plo[:],
                            w2lo[:, 2 * p:2 * p + 2, c0:c0 + HT],
                            s18[:, 2 * p:2 * p + 2, :],
                            start=(p == 0), stop=(p == 3), perf_mode=DR)
                    cols = slice(h * B, (h + 1) * B)
                    nc.vector.scalar_tensor_tensor(
                        m2[:, cols], plo[:], LO_INV, m2[:, cols],
                        AOP.mult, AOP.add)
                    nc.gpsimd.tensor_scalar(s2[0:HT, cols], m2[:, cols],
                                            THRESH, None, AOP.is_gt)

                # layer 4 of step i-1 (deferred so spk3 is long ready)
                if i > 0:
                    l4_block(i - 1)

                # layer-1 psums + LIF for step i+1
                if i < T - 1:
                    l1_block(xh, xl, xc)

                # layer-2 state update (off critical path)
                nc.vector.scalar_tensor_tensor(m2[:], m2[:], BETA,
                                               s2f[0:HT, :],
                                               AOP.mult, AOP.subtract)

                # layer-1 state update for step i+1 — emitted here (not at
                # the loop tail) so the DVE queue tail before the next
                # iteration's psum adds stays short
                if i < T - 1:
                    nc.vector.scalar_tensor_tensor(m1[:], m1[:], BETA,
                                                   s1f[0:HT, :],
                                                   AOP.mult, AOP.subtract)

                # fp8 spike image for step i+1's residual matmuls
                if i < T - 1:
                    nc.scalar.copy(s18[0:HT, :, :], s1f[0:HT, :])

                # layer 3 of step i: single fp32r term, K=127 chunks into
                # one psum; ACT stages psum->SBUF, GPSIMD adds/thresholds
                p3 = ps34.tile([H3, B], F32, tag="p34", name="p3")
                for k in range(NH):
                    nc.tensor.matmul(
                        p3[:],
                        w3c[:, k * H3:(k + 1) * H3],
                        s2[0:HT + 2, k * B:(k + 1) * B],
                        start=(k == 0), stop=(k == NH - 1))
                nc.scalar.copy(cur3[:], p3[:])
                nc.gpsimd.tensor_tensor(m3[:], cur3[:], m3[:], AOP.add)
                nc.gpsimd.tensor_scalar(s3[0:H3, :], m3[:], THRESH, None,
                                        AOP.is_gt)
                nc.vector.scalar_tensor_tensor(m3[:], m3[:], BETA,
                                               s3f[0:H3, :],
                                               AOP.mult, AOP.subtract)

            # ---- epilogue ----
            l4_block(T - 1)

    nc.compile()
    return nc


_CACHE = {}


def _get_nc():
    if "nc" not in _CACHE:
        _CACHE["nc"] = build_bass()
    return _CACHE["nc"]


def _rne12(a):
    """Round fp32 to 12 significand bits (the fp32r grid), RNE —
    bit-identical to the device's fp32r rounding."""
    drop = np.uint64(12)
    u = np.ascontiguousarray(a, np.float32).view(np.uint32).astype(np.uint64)
    half = np.uint64(1 << 11)
    lsb = (u >> drop) & np.uint64(1)
    u2 = ((u + half - np.uint64(1) + lsb) >> drop << drop)
    return u2.astype(np.uint32).view(np.float32).reshape(a.shape)


def _hilo(a):
    hi = _rne12(a)
    lo = _rne12(np.asarray(a, np.float32) - hi)
    return hi, lo


FP8NP = ml_dtypes.float8_e4m3fn


def _prep_inputs(x, w1, b1, w2, b2, w3, b3, w4, b4):
    x = np.ascontiguousarray(x, np.float32)
    # xs[t, f, b_global]; step t of the reference reads x[:, f*T + t]
    xt = np.ascontiguousarray(
        np.transpose(x.reshape(BATCH, F, T), (2, 1, 0)))   # [T, F, BATCH]
    xth, xtl = _hilo(xt)

    w1T = np.ascontiguousarray(w1.T.astype(np.float32))    # [129, 1000]
    w1h, w1l = _hilo(w1T[:128])
    whL, wlL = _hilo(w1T[128])
    b1h, b1l = _hilo(b1.astype(np.float32))
    w1c = np.stack([whL, whL, wlL, b1h, b1l])              # [5, 1000]

    # layer 2: fp32r hi + e4m3 residual * 2^20 (DoubleRow layout)
    w2T = np.ascontiguousarray(w2.T.astype(np.float32))    # [1000, 1000]
    b2 = np.asarray(b2, np.float32)
    w2T_h = _rne12(w2T)
    b2h = _rne12(b2)
    w2hd = np.zeros((NH, HT + 1, H), np.float32)
    for k in range(NH):
        w2hd[k, :HT] = w2T_h[k * HT:(k + 1) * HT]
    w2hd[NH - 1, HT] = b2h
    w2lo = np.zeros((128, NH, 1024), FP8NP)
    resid = (w2T - w2T_h) * np.float32(LO_SCALE)
    b2r = ((b2 - b2h) * np.float32(LO_SCALE)).astype(FP8NP)
    for k in range(NH):
        for h in range(NH):
            w2lo[:HT, k, h * 128:h * 128 + HT] = (
                resid[k * HT:(k + 1) * HT, h * HT:(h + 1) * HT].astype(FP8NP))
            if k == NH - 1:
                w2lo[HT, k, h * 128:h * 128 + HT] = b2r[h * HT:(h + 1) * HT]

    # layer 3: single fp32r term, bias hi/lo in chunk-7 rows 125/126
    w3T = np.ascontiguousarray(w3.T.astype(np.float32))    # [1000, 20]
    b3h, b3l = _hilo(b3.astype(np.float32))
    w3cd = np.zeros((NH, HT + 2, H3), np.float32)
    for k in range(NH):
        w3cd[k, :HT] = _rne12(w3T[k * HT:(k + 1) * HT])
    w3cd[NH - 1, HT] = b3h
    w3cd[NH - 1, HT + 1] = b3l

    # layer 4: single fp32r term + bias hi/lo rows
    w4T = w4.T.astype(np.float32)                          # [20, 10]
    b4h, b4l = _hilo(b4.astype(np.float32))
    w4cd = np.zeros((H3 + 2, H4), np.float32)
    w4cd[0:H3] = _rne12(w4T)
    w4cd[H3] = b4h
    w4cd[H3 + 1] = b4l

    # fp8 spike-tile static rows: row 125 ones (bias fold), 126/127 zero
    s8i = np.zeros((3, NH, B), FP8NP)
    s8i[0] = FP8NP(1.0)

    in_maps = []
    for c in range(NCORES):
        xc = np.empty((T, XR, B), np.float32)
        xc[:, 0:128, :] = xth[:, 0:128, c * B:(c + 1) * B]
        xc[:, 128:256, :] = xtl[:, 0:128, c * B:(c + 1) * B]
        xc[:, 256, :] = xth[:, 128, c * B:(c + 1) * B]
        xc[:, 257, :] = xtl[:, 128, c * B:(c + 1) * B]
        xc[:, 258, :] = xth[:, 128, c * B:(c + 1) * B]
        xc[:, 259, :] = 1.0
        xc[:, 260, :] = 1.0
        in_maps.append({
            "x_aug": xc, "w1h": w1h, "w1l": w1l, "w1c": w1c,
            "w2h": w2hd, "w2lo8": w2lo, "w3c": w3cd, "w4c": w4cd,
            "s18init": s8i,
        })
    return in_maps


def _gather(results):
    spk = np.concatenate(
        [np.transpose(r["out_s"], (0, 2, 1)) for r in results], axis=1)
    mem = np.concatenate(
        [np.transpose(r["out_m"], (0, 2, 1)) for r in results], axis=1)
    return spk, mem


def kernel(x, w1, b1, w2, b2, w3, b3, w4, b4, _trace=False, _trace_kwargs=None):
    # accept numpy or jax arrays, any float dtype
    x, w1, b1, w2, b2, w3, b3, w4, b4 = (
        np.asarray(a, dtype=np.float32)
        for a in (x, w1, b1, w2, b2, w3, b3, w4, b4))
    nc = _get_nc()
    in_maps = _prep_inputs(x, w1, b1, w2, b2, w3, b3, w4, b4)
    res = run_bass_kernel_spmd(
        nc, in_maps, core_ids=list(range(NCORES)),
        trace=_trace, **(_trace_kwargs or {}))
    out = _gather(res.results)
    if _trace:
        return out, res
    return out
